# revision 2
# baseline (speedup 1.0000x reference)
"""CIN (Compressed Interaction Network) kernel for Trainium2, 8-core data parallel.

Math (per batch row b, embedding dim d — R = B*D independent rows):
  layer k: cur_k[m, (b,d)] = sum_{f,g} W_k[f*G+g, m] * x0[f,(b,d)] * x_{k}[g,(b,d)]
  output  = concat_k( sum_d cur_k )    -> [B, 384]

Device strategy (per core, batch-sharded B/8 = 256 -> R = 4096 rows):
  * Everything lives feature-on-partitions: xT [39, R], cur_k^T [128, R].
  * z_k^T [(f,g), R] is built k-tile by k-tile on DVE:
      z-tile_f = cur_{k-1}^T (in0) * bcast(xT[f, :]) (in1)
    where the broadcast tile is materialized by a DRAM->SBUF DMA with a
    0-stride AP (partition broadcast from HBM) — DVE itself cannot
    partition-broadcast.
  * The (f,g) contraction runs as standard PSUM-accumulated matmuls with the
    weight k-tiles stationary, so the f-sum is free.
  * Layer 0's z factors (both sides are x0, k-dim = 39*39 = 1521 dense) are
    host-gathered into two pattern tensors zin0/zin1 [1536, R] (pure indexing,
    no host arithmetic), multiplied on DVE in one shot per chunk.
  * bf16 for z/weights (DVE tensor_tensor 2x mode + smaller DMA), fp32 PSUM.
"""

import os
import sys

sys.path.insert(0, "/opt/trn_rl_repo")

import numpy as np
import ml_dtypes

import concourse.bass as bass
import concourse.mybir as mybir
from concourse import bacc
from concourse.tile import TileContext
from concourse.bass_utils import run_bass_kernel_spmd

BF16 = ml_dtypes.bfloat16

B, F0, D = 2048, 39, 16
M = 128                      # layer width (all three layers)
NCORES = 8
BPC = B // NCORES            # batch per core = 256
R = BPC * D                  # rows per core = 4096
K0 = F0 * F0                 # 1521
K0P = 1536                   # padded to 12 k-tiles
NKT0 = K0P // 128            # 12
NKT = (F0 * M) // 128        # 39 k-tiles for layers 1/2

L = 512                      # bd-chunk (32 b x 16 d)
NCHUNK = R // L              # 8
BPCH = L // D                # 32 batches per chunk

DT = mybir.dt.bfloat16
DTF = mybir.dt.float32

_CACHE = {}


def _build_program():
    nc = bacc.Bacc("TRN2", target_bir_lowering=False, debug=False,
                   num_devices=NCORES)

    xT = nc.declare_dram_parameter("xT", [F0, R], DT, isOutput=False)
    zin0 = nc.declare_dram_parameter("zin0", [K0P, R], DT, isOutput=False)
    zin1 = nc.declare_dram_parameter("zin1", [K0P, R], DT, isOutput=False)
    w0 = nc.declare_dram_parameter("w0", [K0P, M], DT, isOutput=False)
    w1 = nc.declare_dram_parameter("w1", [F0 * M, M], DT, isOutput=False)
    w2 = nc.declare_dram_parameter("w2", [F0 * M, M], DT, isOutput=False)
    ident = nc.declare_dram_parameter("ident", [128, 128], DTF, isOutput=False)
    out = nc.declare_dram_parameter("out", [BPC, 3 * M], DTF, isOutput=True)

    with TileContext(nc) as tc:
        with (
            tc.tile_pool(name="wpool", bufs=1) as wpool,
            tc.tile_pool(name="bcast", bufs=2) as bcpool,
            tc.tile_pool(name="zin", bufs=3) as zinpool,
            tc.tile_pool(name="zt", bufs=3) as zpool,
            tc.tile_pool(name="cur", bufs=2) as curpool,
            tc.tile_pool(name="outp", bufs=1) as outpool,
            tc.tile_pool(name="psum", bufs=4, space="PSUM") as pspool,
            tc.tile_pool(name="pst", bufs=2, space="PSUM") as pstpool,
        ):
            # ---- weights, loaded once, k-tile t at [:, t, :] ----
            w0s = wpool.tile([128, NKT0, M], DT, tag="w0")
            nc.sync.dma_start(out=w0s[:], in_=w0.rearrange("(t p) m -> p t m", p=128))
            w1s = wpool.tile([128, NKT, M], DT, tag="w1")
            nc.sync.dma_start(out=w1s[:], in_=w1.rearrange("(t p) m -> p t m", p=128))
            w2s = wpool.tile([128, NKT, M], DT, tag="w2")
            nc.sync.dma_start(out=w2s[:], in_=w2.rearrange("(t p) m -> p t m", p=128))
            ids = wpool.tile([128, 128], DTF, tag="ident")
            nc.sync.dma_start(out=ids[:], in_=ident[:])

            # per-layer output accumulators [128 m, BPC] fp32
            outacc = [
                outpool.tile([128, BPC], DTF, tag=f"oacc{k}", name=f"oacc{k}")
                for k in range(3)
            ]

            # layer-0 k-tile groups (sizes summing to NKT0) and layer-1/2
            # f-groups (sizes summing to F0): one DVE tensor_tensor per group.
            G0 = [6, 6]
            G12 = [8, 8, 8, 8, 7]

            for c in range(NCHUNK):
                cs = c * L

                # ---------- broadcast tiles: bc[:, f, :] = xT[f, chunk] ----------
                bc = bcpool.tile([128, F0, L], DT, tag="bc")
                for f in range(F0):
                    nc.sync.dma_start(
                        out=bc[:, f, :],
                        in_=xT[f : f + 1, cs : cs + L].to_broadcast((128, L)),
                    )

                # ---------- layer 0 ----------
                ps0 = pspool.tile([128, L], DTF, tag="ps")
                kt = 0
                for gi, gsz in enumerate(G0):
                    zin0t = zinpool.tile([128, gsz, L], DT, tag="zin0")
                    zin1t = zinpool.tile([128, gsz, L], DT, tag="zin1")
                    nc.sync.dma_start(
                        out=zin0t[:],
                        in_=zin0.rearrange("(t p) r -> p t r", p=128)[
                            :, kt : kt + gsz, cs : cs + L
                        ],
                    )
                    nc.sync.dma_start(
                        out=zin1t[:],
                        in_=zin1.rearrange("(t p) r -> p t r", p=128)[
                            :, kt : kt + gsz, cs : cs + L
                        ],
                    )
                    z0t = zpool.tile([128, gsz, L], DT, tag="z")
                    nc.vector.tensor_mul(z0t[:], zin0t[:], zin1t[:])
                    for j in range(gsz):
                        nc.tensor.matmul(
                            ps0[:],
                            w0s[:, kt + j, :],
                            z0t[:, j, :],
                            start=(kt + j == 0),
                            stop=(kt + j == NKT0 - 1),
                        )
                    kt += gsz

                cur0 = curpool.tile([128, L], DT, tag="cur0")
                nc.scalar.copy(cur0[:], ps0[:])
                nc.vector.tensor_reduce(
                    outacc[0][:, c * BPCH : (c + 1) * BPCH],
                    ps0[:].rearrange("p (b d) -> p b d", d=D),
                    axis=mybir.AxisListType.X,
                    op=mybir.AluOpType.add,
                )

                # ---------- layers 1 and 2 ----------
                prev = cur0
                for lyr, ws in ((1, w1s), (2, w2s)):
                    ps = pspool.tile([128, L], DTF, tag="ps")
                    f = 0
                    for gsz in G12:
                        zt = zpool.tile([128, gsz, L], DT, tag="z")
                        nc.vector.tensor_mul(
                            zt[:],
                            prev[:].unsqueeze(1).to_broadcast((128, gsz, L)),
                            bc[:, f : f + gsz, :],
                        )
                        for j in range(gsz):
                            nc.tensor.matmul(
                                ps[:],
                                ws[:, f + j, :],
                                zt[:, j, :],
                                start=(f + j == 0),
                                stop=(f + j == F0 - 1),
                            )
                        f += gsz

                    nc.vector.tensor_reduce(
                        outacc[lyr][:, c * BPCH : (c + 1) * BPCH],
                        ps[:].rearrange("p (b d) -> p b d", d=D),
                        axis=mybir.AxisListType.X,
                        op=mybir.AluOpType.add,
                    )
                    if lyr == 1:
                        cur1 = curpool.tile([128, L], DT, tag="cur1")
                        nc.scalar.copy(cur1[:], ps[:])
                        prev = cur1

            # ---------- transpose [128 m, BPC b] -> [BPC, 128] and store ----------
            for k in range(3):
                for h in range(BPC // 128):
                    pst = pstpool.tile([128, 128], DTF, tag="pst")
                    nc.tensor.transpose(
                        pst[:], outacc[k][:, h * 128 : (h + 1) * 128], ids[:]
                    )
                    ot = curpool.tile([128, 128], DTF, tag="otile")
                    nc.scalar.copy(ot[:], pst[:])
                    nc.sync.dma_start(
                        out=out[h * 128 : (h + 1) * 128, k * M : (k + 1) * M],
                        in_=ot[:],
                    )

    nc.compile()
    return nc


def _host_prep(inputs, f0, f1, f2):
    """Per-core input maps. Pure layout/casting, no arithmetic."""
    x = np.asarray(inputs)
    # weights, shared across cores
    w0 = np.zeros((K0P, M), dtype=BF16)
    w0[:K0, :] = np.asarray(f0).astype(BF16)
    w1 = np.asarray(f1).astype(BF16)
    w2 = np.asarray(f2).astype(BF16)
    ident = np.eye(128, dtype=np.float32)

    # layer-0 z-factor gather patterns (f-major dense k = f*39+g, padded)
    pidx = np.arange(K0P)
    fidx = np.minimum(pidx // F0, F0 - 1)
    gidx = pidx % F0
    valid = (pidx < K0).astype(BF16)[:, None]

    maps = []
    for c in range(NCORES):
        xs = x[c * BPC : (c + 1) * BPC]                    # [256, 39, 16]
        xT = np.ascontiguousarray(xs.transpose(1, 0, 2).reshape(F0, R)).astype(BF16)
        zin0 = xT[gidx] * valid                            # [1536, R]
        zin1 = xT[fidx] * valid
        maps.append(
            dict(xT=xT, zin0=zin0, zin1=zin1, w0=w0, w1=w1, w2=w2, ident=ident)
        )
    return maps


def kernel(**inputs) -> np.ndarray:
    if "nc" not in _CACHE:
        _CACHE["nc"] = _build_program()
    nc = _CACHE["nc"]
    maps = _host_prep(inputs["inputs"], inputs["f0"], inputs["f1"], inputs["f2"])
    res = run_bass_kernel_spmd(nc, maps, list(range(NCORES)))
    return np.concatenate([res.results[c]["out"] for c in range(NCORES)], axis=0)


if __name__ == "__main__":
    rng = np.random.default_rng(0)
    ins = {
        "inputs": rng.standard_normal((B, F0, D), dtype=np.float32),
        "f0": (rng.standard_normal((K0, M)) * 0.05).astype(np.float32),
        "f1": (rng.standard_normal((F0 * M, M)) * 0.05).astype(np.float32),
        "f2": (rng.standard_normal((F0 * M, M)) * 0.05).astype(np.float32),
    }
    out = kernel(**ins)
    print("out", out.shape, out.dtype)


# revision 3
# speedup vs baseline: 2.1700x; 2.1700x over previous
"""CIN (Compressed Interaction Network) kernel for Trainium2, 8-core data parallel.

Math (per batch row b, embedding dim d — R = B*D independent rows):
  layer k: cur_k[m, (b,d)] = sum_{f,g} W_k[f*G+g, m] * x0[f,(b,d)] * x_{k}[g,(b,d)]
  output  = concat_k( sum_d cur_k )    -> [B, 384]

Device strategy (per core, batch-sharded B/8 = 256 -> R = 4096 rows):
  * Everything lives feature-on-partitions: cur_k^T [128, R] etc.
  * z_k^T [(f,g), R] is built k-tile by k-tile on DVE tensor_tensor (bf16 2x):
      z-tile_f = cur_{k-1}^T * bcast(x0^T[f, :])
    The broadcast tiles come from one DRAM->SBUF DMA per chunk with a
    0-stride partition AP (DVE cannot partition-broadcast; DMA can).
  * The (f,g) contraction is standard PSUM-accumulated matmuls with weight
    k-tiles stationary, so the f-sum is free.
  * Layer 0 uses the x (x) x symmetry: W0 is host-symmetrized to the upper
    triangle (k: 1521 -> 780, padded 896) and the two z factors are
    host-gathered index patterns of x^T (pure indexing, no arithmetic).
  * All DRAM operands are laid out chunk-major on the host so every DMA has
    large contiguous per-partition runs (big packets -> full DMA bandwidth).
"""

import sys

sys.path.insert(0, "/opt/trn_rl_repo")

import numpy as np
import ml_dtypes

import concourse.bass as bass
import concourse.mybir as mybir
from concourse import bacc
from concourse.tile import TileContext
from concourse.bass_utils import run_bass_kernel_spmd

BF16 = ml_dtypes.bfloat16

B, F0, D = 2048, 39, 16
M = 128                      # layer width (all three layers)
NCORES = 8
BPC = B // NCORES            # batch per core = 256
R = BPC * D                  # rows per core = 4096
K0 = (F0 * (F0 + 1)) // 2    # 780 (triangular)
K0P = 896                    # padded to 7 k-tiles
NKT0 = K0P // 128            # 7
NKT = (F0 * M) // 128        # 39 k-tiles for layers 1/2

L = 512                      # bd-chunk (32 b x 16 d)
NCHUNK = R // L              # 8
BPCH = L // D                # 32 batches per chunk

DT = mybir.dt.bfloat16
DTF = mybir.dt.float32

_CACHE = {}


def _build_program():
    nc = bacc.Bacc("TRN2", target_bir_lowering=False, debug=False,
                   num_devices=NCORES)

    # chunk-major layouts so per-partition DMA runs are contiguous
    xT = nc.declare_dram_parameter("xT", [NCHUNK, F0 * L], DT, isOutput=False)
    zin0 = nc.declare_dram_parameter("zin0", [NCHUNK, 128, NKT0, L], DT,
                                     isOutput=False)
    zin1 = nc.declare_dram_parameter("zin1", [NCHUNK, 128, NKT0, L], DT,
                                     isOutput=False)
    w0 = nc.declare_dram_parameter("w0", [K0P, M], DT, isOutput=False)
    w1 = nc.declare_dram_parameter("w1", [F0 * M, M], DT, isOutput=False)
    w2 = nc.declare_dram_parameter("w2", [F0 * M, M], DT, isOutput=False)
    ident = nc.declare_dram_parameter("ident", [128, 128], DTF, isOutput=False)
    out = nc.declare_dram_parameter("out", [BPC, 3 * M], DTF, isOutput=True)

    with TileContext(nc) as tc:
        with (
            tc.tile_pool(name="wpool", bufs=1) as wpool,
            tc.tile_pool(name="bcast", bufs=2) as bcpool,
            tc.tile_pool(name="zin", bufs=2) as zinpool,
            tc.tile_pool(name="zt", bufs=3) as zpool,
            tc.tile_pool(name="cur", bufs=2) as curpool,
            tc.tile_pool(name="outp", bufs=1) as outpool,
            tc.tile_pool(name="psum", bufs=4, space="PSUM") as pspool,
            tc.tile_pool(name="pst", bufs=2, space="PSUM") as pstpool,
        ):
            # ---- weights, loaded once, k-tile t at [:, t, :] ----
            w0s = wpool.tile([128, NKT0, M], DT, tag="w0")
            nc.sync.dma_start(out=w0s[:], in_=w0.rearrange("(t p) m -> p t m", p=128))
            w1s = wpool.tile([128, NKT, M], DT, tag="w1")
            nc.sync.dma_start(out=w1s[:], in_=w1.rearrange("(t p) m -> p t m", p=128))
            w2s = wpool.tile([128, NKT, M], DT, tag="w2")
            nc.sync.dma_start(out=w2s[:], in_=w2.rearrange("(t p) m -> p t m", p=128))
            ids = wpool.tile([128, 128], DTF, tag="ident")
            nc.sync.dma_start(out=ids[:], in_=ident[:])

            # per-layer output accumulators [128 m, BPC] fp32
            outacc = [
                outpool.tile([128, BPC], DTF, tag=f"oacc{k}", name=f"oacc{k}")
                for k in range(3)
            ]

            G0 = [4, 3]          # layer-0 k-tile TT groups (sum NKT0)
            G12 = [8, 8, 8, 8, 7]  # layer-1/2 f-groups (sum F0)

            for c in range(NCHUNK):
                # ---------- broadcast set: bc[:, f, :] = x^T[f, chunk] ----------
                bc = bcpool.tile([128, F0, L], DT, tag="bc")
                nc.sync.dma_start(
                    out=bc[:],
                    in_=xT[c : c + 1, :].to_broadcast((128, F0 * L)).rearrange(
                        "p (f l) -> p f l", f=F0
                    ),
                )

                # ---------- layer 0 (triangular) ----------
                zin0t = zinpool.tile([128, NKT0, L], DT, tag="zin0")
                zin1t = zinpool.tile([128, NKT0, L], DT, tag="zin1")
                nc.scalar.dma_start(out=zin0t[:], in_=zin0[c])
                nc.scalar.dma_start(out=zin1t[:], in_=zin1[c])

                ps0 = pspool.tile([128, L], DTF, tag="ps")
                kt = 0
                for gsz in G0:
                    z0t = zpool.tile([128, gsz, L], DT, tag="z")
                    nc.vector.tensor_mul(
                        z0t[:],
                        zin0t[:, kt : kt + gsz, :],
                        zin1t[:, kt : kt + gsz, :],
                    )
                    for j in range(gsz):
                        nc.tensor.matmul(
                            ps0[:],
                            w0s[:, kt + j, :],
                            z0t[:, j, :],
                            start=(kt + j == 0),
                            stop=(kt + j == NKT0 - 1),
                        )
                    kt += gsz

                cur0 = curpool.tile([128, L], DT, tag="cur0")
                nc.scalar.copy(cur0[:], ps0[:])
                nc.vector.tensor_reduce(
                    outacc[0][:, c * BPCH : (c + 1) * BPCH],
                    ps0[:].rearrange("p (b d) -> p b d", d=D),
                    axis=mybir.AxisListType.X,
                    op=mybir.AluOpType.add,
                )

                # ---------- layers 1 and 2 ----------
                prev = cur0
                for lyr, ws in ((1, w1s), (2, w2s)):
                    ps = pspool.tile([128, L], DTF, tag="ps")
                    f = 0
                    for gsz in G12:
                        zt = zpool.tile([128, gsz, L], DT, tag="z")
                        nc.vector.tensor_mul(
                            zt[:],
                            prev[:].unsqueeze(1).to_broadcast((128, gsz, L)),
                            bc[:, f : f + gsz, :],
                        )
                        for j in range(gsz):
                            nc.tensor.matmul(
                                ps[:],
                                ws[:, f + j, :],
                                zt[:, j, :],
                                start=(f + j == 0),
                                stop=(f + j == F0 - 1),
                            )
                        f += gsz

                    nc.vector.tensor_reduce(
                        outacc[lyr][:, c * BPCH : (c + 1) * BPCH],
                        ps[:].rearrange("p (b d) -> p b d", d=D),
                        axis=mybir.AxisListType.X,
                        op=mybir.AluOpType.add,
                    )
                    if lyr == 1:
                        cur1 = curpool.tile([128, L], DT, tag="cur1")
                        nc.scalar.copy(cur1[:], ps[:])
                        prev = cur1

            # ---------- transpose [128 m, BPC b] -> [BPC, 128] and store ----------
            for k in range(3):
                for h in range(BPC // 128):
                    pst = pstpool.tile([128, 128], DTF, tag="pst")
                    nc.tensor.transpose(
                        pst[:], outacc[k][:, h * 128 : (h + 1) * 128], ids[:]
                    )
                    ot = curpool.tile([128, 128], DTF, tag="otile")
                    nc.scalar.copy(ot[:], pst[:])
                    nc.sync.dma_start(
                        out=out[h * 128 : (h + 1) * 128, k * M : (k + 1) * M],
                        in_=ot[:],
                    )

    nc.compile()
    return nc


def _host_prep(inputs, f0, f1, f2):
    """Per-core input maps. Pure layout/cast/index-gather, no FLOP offload
    (except the W0 symmetrization, which is weight preprocessing)."""
    x = np.asarray(inputs)

    # symmetrized triangular W0: rows (f, g) f<=g
    f0n = np.asarray(f0).reshape(F0, F0, M)
    fi, gi = np.triu_indices(F0)
    w0t = f0n[fi, gi] + np.where((fi != gi)[:, None], f0n[gi, fi], 0.0)
    w0 = np.zeros((K0P, M), dtype=BF16)
    w0[:K0] = w0t.astype(BF16)

    w1 = np.asarray(f1).astype(BF16)
    w2 = np.asarray(f2).astype(BF16)
    ident = np.eye(128, dtype=np.float32)

    # layer-0 z-factor gather indices (triangular, k-row = tile*128 + p)
    pidx = np.arange(K0P)
    fidx = np.zeros(K0P, np.int64)
    gidx = np.zeros(K0P, np.int64)
    fidx[:K0], gidx[:K0] = fi, gi
    valid = (pidx < K0).astype(BF16)[:, None]

    maps = []
    for c in range(NCORES):
        xs = x[c * BPC : (c + 1) * BPC]                    # [256, 39, 16]
        xTf = np.ascontiguousarray(
            xs.transpose(1, 0, 2).reshape(F0, R)
        ).astype(BF16)                                     # [39, R]
        # chunk-major broadcast source: [NCHUNK, F0*L]
        xTc = np.ascontiguousarray(
            xTf.reshape(F0, NCHUNK, L).transpose(1, 0, 2)
        ).reshape(NCHUNK, F0 * L)
        # layer-0 factors [K0P, R] -> chunk-major [NCHUNK, 128, NKT0, L]
        z0a = (xTf[gidx] * valid).reshape(NKT0, 128, NCHUNK, L)
        z0b = (xTf[fidx] * valid).reshape(NKT0, 128, NCHUNK, L)
        zin0c = np.ascontiguousarray(z0a.transpose(2, 1, 0, 3))
        zin1c = np.ascontiguousarray(z0b.transpose(2, 1, 0, 3))
        maps.append(
            dict(xT=xTc, zin0=zin0c, zin1=zin1c, w0=w0, w1=w1, w2=w2,
                 ident=ident)
        )
    return maps


def kernel(**inputs) -> np.ndarray:
    if "nc" not in _CACHE:
        _CACHE["nc"] = _build_program()
    nc = _CACHE["nc"]
    maps = _host_prep(inputs["inputs"], inputs["f0"], inputs["f1"], inputs["f2"])
    res = run_bass_kernel_spmd(nc, maps, list(range(NCORES)))
    return np.concatenate([res.results[c]["out"] for c in range(NCORES)], axis=0)


if __name__ == "__main__":
    rng = np.random.default_rng(0)
    ins = {
        "inputs": rng.standard_normal((B, F0, D), dtype=np.float32),
        "f0": (rng.standard_normal((F0 * F0, M)) * 0.05).astype(np.float32),
        "f1": (rng.standard_normal((F0 * M, M)) * 0.05).astype(np.float32),
        "f2": (rng.standard_normal((F0 * M, M)) * 0.05).astype(np.float32),
    }
    out = kernel(**ins)
    print("out", out.shape, out.dtype)
